# revision 21
# baseline (speedup 1.0000x reference)
"""ClusterMemory teacher loss kernel for 8x Trainium2 NeuronCores.

Strategy (tensor-parallel over the cluster/num_samples axis, per the
sharding hint): each of the 8 cores holds a 1024-row shard of each of the
three feature banks, computes A = -2 * x_hat @ f_shard^T on the tensor
engine (scale folded into the stationary operand), and reduces each row of
the [256, 1024] shard to three partial statistics:

  L1 = sum_j exp(20 * s)          (CE#1 logsumexp partial)
  U1 = sum_j exp(d)               (softmax(d) normalizer partial)
  U2 = sum_j exp(2d)              (2nd moment for the CE#2 logsumexp)

with d = sqrt(x2 + f2 - 2s) computed as exp(0.5*ln(.)) so that every
activation (Exp/Ln/Identity/Copy) lives in the single
`natural_log_exp_and_others` table set (no ~2.7us table reloads).

Host (fp64) combine: CE1 = mean_b [log(sum_c L1) - 20*s_t]; for CE2 the
log-sum-exp of the softmax probabilities u_j/E is expanded as
  sum_j exp(u_j/E) = N + 1 + (sum u^2)/(2 E^2) + O(1e-12)
(u_j/E <= ~3e-4, so the dropped cubic term is ~1e-12 relative), and the
target-entry terms s_t, u_t are evaluated exactly on the host from the
fp32 inputs. No collectives are needed; per-core outputs are 9 KiB.
"""

import numpy as np
import ml_dtypes

import concourse.bass as bass
import concourse.mybir as mybir
import concourse.tile as tile
from concourse import bacc
from concourse.bass_utils import run_bass_kernel_spmd

B = 256          # batch
D = 2048         # feature dim
N = 8192         # cluster count (total)
NCORES = 8
NSH = N // NCORES  # 1024 cluster rows per core
KT = D // 128      # 16 contraction chunks
MT = B // 128      # 2 partition tiles of the batch
JT = NSH // 512    # 2 matmul free-dim chunks
TEMP = 0.05
EPS = 1e-12
LAMBDA2 = 0.5
SCOL = 14  # stats columns per branch: [L1_j0, L1_j1, bn(v)_j0(6), bn(v)_j1(6)]

# u = exp(sqrt(z)) on z in [1.55, 2.45] as a nested-square polynomial
#   u = (PC*(z + PB)^2 + PD)^2     (two ACT `Square` passes; max rel err
# 1.2e-4, which cancels to <1e-7 in the softmax-CE because the same
# polynomial is used for the host-side target term and softmax is
# invariant to the common scale / first-order distortion).
PB = np.float32(-15.160572726694888)
PC = np.float32(-0.013651339885605563)
PD = np.float32(4.392563556355194)


def _poly_u(z):
    """Same u-polynomial as the device epilogue (fp64 on fp32 consts)."""
    r = float(PC) * (z + float(PB)) ** 2 + float(PD)
    return r * r

F32 = mybir.dt.float32

# mm dtype config: (mybir dtype, numpy dtype, range prescale)
_MM_CONFIGS = {
    "bf16": (mybir.dt.bfloat16, ml_dtypes.bfloat16, 1.0),
    "fp8": (mybir.dt.float8e4, ml_dtypes.float8_e4m3, 8.0),
}
import os as _os
MM_MODE = _os.environ.get("KMM_MODE", "fp8")
DOUBLE_ROW = _os.environ.get("KDR", "1") == "1"  # fp8 DoubleRow perf mode
DVE_SUMS = _os.environ.get("KDVE", "1") == "1"   # u/u^2 row-sums on DVE vs ACT

_cache = {}


class _only_combined_act_set:
    """Restrict the activation-table chooser to `natural_log_exp_and_others`
    during our compile: the greedy first-match chooser would otherwise bounce
    between `exp_and_others` and `natural_log` (one ~2.7us table load per
    switch). Emptying the other sets (instead of reordering) keeps
    act_func_set_id aligned with act_info.json indices."""

    def __enter__(self):
        self._orig = bacc.get_activation_tables
        orig = self._orig

        def patched(arch):
            tables = orig(arch)
            return {
                name: (funcs if name == "natural_log_exp_and_others" else set())
                for name, funcs in tables.items()
            }

        bacc.get_activation_tables = patched
        return self

    def __exit__(self, *exc):
        bacc.get_activation_tables = self._orig
        return False


def _build_nc(mode):
    mm_dt, _, sc = _MM_CONFIGS[mode]
    q = 1.0 / (sc * sc)  # descale for the psum values
    AF = mybir.ActivationFunctionType

    nc = bacc.Bacc(
        "TRN2",
        target_bir_lowering=False,
        debug=False,
        enable_asserts=False,
        num_devices=NCORES,
    )

    xt = nc.dram_tensor("xt", [3, 128, KT, B], mm_dt, kind="ExternalInput")
    ft = nc.dram_tensor("ft", [3, 128, KT, NSH], mm_dt, kind="ExternalInput")
    x2 = nc.dram_tensor("x2", [128, 3 * MT], F32, kind="ExternalInput")
    stats = nc.dram_tensor("stats", [MT, 128, 3 * SCOL], F32, kind="ExternalOutput")

    with tile.TileContext(nc) as tc:
        with (
            tc.tile_pool(name="xtp", bufs=2) as xt_pool,
            tc.tile_pool(name="ftp", bufs=8) as ft_pool,
            tc.tile_pool(name="x2p", bufs=1) as x2_pool,
            tc.tile_pool(name="scr", bufs=8) as scr_pool,
            tc.tile_pool(name="stp", bufs=1) as st_pool,
            tc.tile_pool(name="ps", bufs=8, space="PSUM") as psum_pool,
        ):
            stats_sb = []
            for m in range(MT):
                st_t = st_pool.tile([128, 3 * SCOL], F32, name=f"st{m}", tag=f"st{m}")
                stats_sb.append(st_t)

            use_dr = DOUBLE_ROW and mode == "fp8"
            kstep = 2 if use_dr else 1
            perf_mode = mybir.MatmulPerfMode.DoubleRow if use_dr else None
            KQ = 4           # ft quarter-chunks
            NCH = KT // KQ

            # per-row x2 scalars in one small DMA up front
            x2t = x2_pool.tile([128, 3 * MT], F32, name="x2t", tag="x2t")
            nc.sync.dma_start(out=x2t, in_=x2[:, :])
            # Square-activation per-partition bias: x2 + f2(=1) + PB
            bias6 = x2_pool.tile([128, 3 * MT], F32, name="bias6", tag="bias6")
            nc.vector.tensor_scalar(
                out=bias6, in0=x2t, scalar1=float(1.0 + float(PB)), scalar2=None,
                op0=mybir.AluOpType.add,
            )

            def epilogue(ps_j, st_t, br, m, j):
                """Reduce one [128, 512] psum slice into L1/U1 stat columns."""
                c0 = SCOL * br
                col = 2 * br + m
                junk = scr_pool.tile([128, 512], F32,
                                     name=f"junk_{br}_{m}_{j}", tag="junk")
                # L1 partial: sum_j exp(20 s) = sum_j exp(-10 * q * A)
                nc.scalar.activation(
                    junk, ps_j, AF.Exp, scale=-10.0 * q,
                    accum_out=st_t[:, c0 + j:c0 + j + 1],
                )
                # s1 = (q*A + (x2 + 1 + PB))^2 = (d2 + PB)^2
                s1 = scr_pool.tile([128, 512], F32,
                                   name=f"s1_{br}_{m}_{j}", tag="s1")
                nc.scalar.activation(s1, ps_j, AF.Square, scale=q,
                                     bias=bias6[:, col:col + 1])
                # v = PC*s1 + PD  (so u = v^2);  U1 = sum_j v^2
                v = scr_pool.tile([128, 512], F32, name=f"v_{br}_{m}_{j}", tag="v")
                nc.vector.tensor_scalar(
                    out=v, in0=s1, scalar1=float(PC), scalar2=float(PD),
                    op0=mybir.AluOpType.mult, op1=mybir.AluOpType.add,
                )
                nc.vector.bn_stats(
                    out=st_t[:, c0 + 2 + 6 * j:c0 + 8 + 6 * j], in_=v,
                )

            for br in range(3):
                fks = []
                for h in range(NCH):
                    fk = ft_pool.tile([128, KQ, NSH], mm_dt,
                                      name=f"fk_{br}_{h}", tag="fk")
                    nc.sync.dma_start(
                        out=fk, in_=ft[br, :, h * KQ:(h + 1) * KQ, :])
                    fks.append(fk)
                    if h == 0:
                        xk = xt_pool.tile([128, KT, B], mm_dt,
                                          name=f"xk_{br}", tag="xk")
                        nc.sync.dma_start(out=xk, in_=xt[br])

                # m-outer so ps[m=0]'s accumulation group (and epilogue)
                # completes while m=1's matmuls still run -> only the last
                # m-tile's epilogue is exposed at the kernel tail.
                for m in range(MT):
                    last = (br == 2 and m == MT - 1)
                    pss = [
                        psum_pool.tile([128, 512], F32,
                                       name=f"ps_{br}_{m}_{j}", tag="ps")
                        for j in range(JT)
                    ]

                    def mm(j, k):
                        fk = fks[k // KQ]
                        kk = k % KQ
                        if use_dr:
                            lhsT = xk[:, k:k + 2, m * 128:(m + 1) * 128]
                            rhs = fk[:, kk:kk + 2, j * 512:(j + 1) * 512]
                        else:
                            lhsT = xk[:, k, m * 128:(m + 1) * 128]
                            rhs = fk[:, kk, j * 512:(j + 1) * 512]
                        nc.tensor.matmul(
                            pss[j][:, :], lhsT, rhs,
                            start=(k == 0), stop=(k == KT - kstep),
                            perf_mode=perf_mode,
                        )

                    if not last:
                        # j-inner: one weight load serves both j streams
                        for k in range(0, KT, kstep):
                            for j in range(JT):
                                mm(j, k)
                        for j in range(JT):
                            epilogue(pss[j], stats_sb[m], br, m, j)
                    else:
                        # j-outer on the final tile: the j=0 epilogue
                        # overlaps the j=1 matmuls, shortening the tail
                        for j in range(JT):
                            for k in range(0, KT, kstep):
                                mm(j, k)
                            epilogue(pss[j], stats_sb[m], br, m, j)

            for m in range(MT):
                nc.gpsimd.dma_start(out=stats[m], in_=stats_sb[m])

    with _only_combined_act_set():
        nc.compile()
    return nc


def _get_nc(mode):
    if mode not in _cache:
        _cache[mode] = _build_nc(mode)
    return _cache[mode]


def _prepare_branch(x_raw, f, mode):
    """Host-side prep for one branch. Returns per-core input arrays and the
    fp64 host-side quantities."""
    _, np_dt, sc = _MM_CONFIGS[mode]
    x_raw = np.asarray(x_raw, dtype=np.float32)
    f = np.asarray(f, dtype=np.float32)

    n = np.sqrt(np.sum(x_raw.astype(np.float64) ** 2, axis=1, keepdims=True))
    xh64 = x_raw.astype(np.float64) / np.maximum(n, EPS)
    xh = xh64.astype(np.float32)

    x2 = np.sum(xh.astype(np.float64) ** 2, axis=1)   # [B], ~1.0

    # partition-major [128, KT, cols]: contiguous 4 KB per partition line
    xt = ((-2.0 * sc) * xh.T).astype(np_dt)                       # [D, B]
    xt = np.ascontiguousarray(xt.reshape(KT, 128, B).transpose(1, 0, 2))
    fT = (sc * f.T).astype(np_dt)                                 # [D, N]
    ft_shards = [
        np.ascontiguousarray(
            fT[:, c * NSH:(c + 1) * NSH].reshape(KT, 128, NSH).transpose(1, 0, 2))
        for c in range(NCORES)
    ]
    x2_dev = x2.astype(np.float32).reshape(MT, 128).T  # [128, MT]
    return xt, ft_shards, x2_dev, xh, x2


def _host_combine(stats_by_core, xh, x2, f, targets):
    """stats_by_core: [NCORES] of [MT, 128, SCOL] for this branch.
    Returns the branch loss (fp64)."""
    st = np.stack([s.reshape(B, SCOL) for s in stats_by_core]).astype(np.float64)
    L1 = (st[..., 0] + st[..., 1]).sum(axis=0)   # [B]
    # E = sum v^2 from the bn_stats moments: (cnt, mean, cnt*var) x even/odd
    U1 = np.zeros_like(L1)
    for base in (2, 8):
        for off in (0, 3):
            c = st[..., base + off]
            mn = st[..., base + off + 1]
            cv = st[..., base + off + 2]
            U1 = U1 + cv + c * mn * mn
    E = U1.sum(axis=0)    # [B]

    f_t = np.asarray(f, np.float32)[targets].astype(np.float64)   # [B, D]
    s_t = np.sum(xh.astype(np.float64) * f_t, axis=1)
    # f2 == 1 for the L2-normalized banks (same assumption as the device)
    z_t = np.maximum(x2 + 1.0 - 2.0 * s_t, 0.0)
    u_t = _poly_u(z_t)  # same polynomial as the device (softmax-consistent)

    ce1 = np.mean(np.log(L1) - s_t / TEMP)
    # exact: log(N + 1 + U2/(2E^2)); the U2 term is ~8e-9 relative -> drop
    ce2 = np.log(N + 1.0) - np.mean(u_t / E)
    return ce1 + ce2


def run(inputs, inputs_up, inputs_down, targets, epoch, features, features_up,
        features_down, trace=False):
    mode = MM_MODE
    nc = _get_nc(mode)
    targets = np.asarray(targets).astype(np.int64)

    xs = [inputs, inputs_up, inputs_down]
    fs = [features, features_up, features_down]

    prep = [_prepare_branch(x, f, mode) for x, f in zip(xs, fs)]

    in_maps = []
    for c in range(NCORES):
        in_maps.append({
            "xt": np.stack([p[0] for p in prep]),                 # [3,128,KT,B]
            "ft": np.stack([p[1][c] for p in prep]),              # [3,128,KT,NSH]
            # [128, 3*MT]: column 2*br+m holds x2 of batch rows m*128..m*128+127
            "x2": np.ascontiguousarray(
                np.concatenate([p[2] for p in prep], axis=1)),
        })

    res = run_bass_kernel_spmd(nc, in_maps, list(range(NCORES)), trace=trace)

    branch_losses = []
    for bi in range(3):
        stats_by_core = [
            res.results[c]["stats"][:, :, SCOL * bi:SCOL * (bi + 1)]
            for c in range(NCORES)
        ]
        _, _, _, xh, x2 = prep[bi]
        branch_losses.append(
            _host_combine(stats_by_core, xh, x2,
                          np.asarray(fs[bi], np.float32), targets)
        )

    l_mid, l_up, l_down = branch_losses
    loss = (1.0 - LAMBDA2) * l_mid + LAMBDA2 * (l_up + l_down)
    out = np.float32(loss)
    return (out, res) if trace else out


def kernel(**inputs):
    return run(**inputs)

